# revision 1
# baseline (speedup 1.0000x reference)
"""Sinkhorn distance (entropic OT) on 8 Trainium2 NeuronCores — v2.

Data-parallel over batch: B=16 -> 2 batches/core.  Per batch:

  G = exp(-C/eps) built directly on PE via an augmented 66-row contraction
  (softmaxed x,y transposed + the -x2/2 / -y2/2 folding rows), so the
  multiplicative Sinkhorn scalings start at f0 = g0 = 1 and the first
  row-sum s0 = G@1 falls out of the exp pass (ACT accumulator / an extra
  row-sum for Schraudolph chunks).

  exp is split across engines: ACT computes exact exp chunks (with free
  accum_out row sums); DVE/Pool chunks use a Schraudolph exp (one
  tensor_scalar f32->int16 pass bitcast to bf16; max entry err ~4%, final
  cost err ~5e-4, validated offline).

  GT (needed only for the s1 = G@g1 matvec) is produced by DMA xbar
  transposes (SP engine HWDGE; free of compute engines) and/or PE
  transposes (split tunable).

  All Sinkhorn state lives in column layout [128, 8].  Matvecs are
  tall-skinny accumulating matmuls (dst [128, 1]; the big matrix is the
  stationary operand), so each full matvec costs ~64 tiny instructions
  instead of a 3.4us wide stream.  The final cost contracts through one
  [128 j, 66] Q stream that also yields sigma1 (merging the last
  v-update), then small fused multiply-reduces.

Sync legality: this walrus build accepts only 1 semaphore wait per
instruction.  Instead of manual observe/claim gymnastics, a generic
legalizer wraps TileContext._commit_and_lower and splits any multi-wait
instruction into single-wait EventSemaphore instructions on the same
engine queue ahead of it.
"""

import sys

sys.path.insert(0, "/opt/trn_rl_repo")

import numpy as np

EPS = 0.1
THRESH = 0.1
MAX_ITER = 100
B, N, D = 16, 1024, 64
NCORES = 8
BL = B // NCORES
GP = 8
MU = float(np.float32(1.0 / N + 1e-8))
LOG_MU = float(np.log(np.float32(1.0 / N + 1e-8), dtype=np.float32))
LOG2E = 1.4426950408889634

_CACHE: dict = {}
_PATCHED = [False]

# ---------------- engine assignment knobs (tuned against TimelineSim) ----
# exp engine per (b, ic) chunk index 0..15: 'A' = ACT exact, 'V' = DVE
# Schraudolph, 'P' = Pool Schraudolph.
EXP_ASSIGN = "AAVAAVAA" + "AAVAAVAA"
# s0 row-sum engine for non-ACT chunks: 'V' or 'P'
S0_ASSIGN = {"V": "P", "P": "V"}
# GT stripe (b, g) 0..15: 'D' = DMA transpose, 'E' = PE transpose
GT_ASSIGN = "DDDDDDDD" + "DDDDDDDD"
# PE-built GT stripes: PSUM->SBUF copy engine cycle
GT_COPY = "AV"


def _patch_tile():
    """1) split multi-wait instructions into single-wait EventSemaphores;
    2) split the kernel-tail drain the same way (as the v1 kernel did)."""
    if _PATCHED[0]:
        return
    import concourse.tile as tile
    from concourse import mybir
    from concourse.vector_clock import ScopedClock

    _orig_cal = tile.TileContext._commit_and_lower

    def _cal(self, inst, original_block, old_bb_map, bb_to_exit_bb):
        si = inst.sync_info
        if (si is not None and si.on_wait and len(si.on_wait) > 1
                and inst.engine != mybir.EngineType.Unassigned):
            waits = list(si.on_wait)
            eng = self.nc.engines[inst.engine]
            by_name = {s.name: s for s in self.sems.allocated().values()}
            kept = []
            for w in waits[:-1]:
                sem = by_name.get(w.ant_name)
                if sem is None:
                    kept.append(w)
                    continue
                eng.wait_ge(sem, w.wait_value)
            si.on_wait = kept + [waits[-1]]
        return _orig_cal(self, inst, original_block, old_bb_map, bb_to_exit_bb)

    tile.TileContext._commit_and_lower = _cal

    def _drain_and_barrier(self, tick_clock, wait_clock):
        nc = self.nc
        drain_inst = nc.sync.drain()
        wait_clock.add_sem_waits(
            drain_inst.ins, ScopedClock({None: tick_clock.global_clock}))
        waits = list(drain_inst.ins.sync_info.on_wait or [])
        if len(waits) > 1:
            drain_inst.ins.sync_info.on_wait = waits[:1]
            by_name = {s.name: s for s in self.sems.allocated().values()}
            for w in waits[1:]:
                d2 = nc.sync.drain()
                d2._wait_ge(by_name[w.ant_name], w.wait_value)
        nc.all_engine_barrier()
        assert self.sems is not None
        popped = nc._tile_sem_poison_stack.pop()
        assert popped is self._sem_poison
        nc.clear_and_free_semaphores(list(self.sems.allocated().values()))
        nc.all_engine_barrier()

    tile.TileContext._drain_and_barrier = _drain_and_barrier
    _PATCHED[0] = True


HEAT_N = 0


def _build_program(T1: int):
    import concourse.bass as bass
    import concourse.tile as tile
    from concourse import mybir

    _patch_tile()

    f32 = mybir.dt.float32
    bf16 = mybir.dt.bfloat16
    i16 = mybir.dt.int16
    AF = mybir.ActivationFunctionType
    X = mybir.AxisListType.X
    ALU = mybir.AluOpType

    nc = bass.Bass("TRN2", target_bir_lowering=False, debug=False,
                   num_devices=NCORES, num_swdge_queues=1)

    # inputs: host-prepacked [128, 2BL, GP, D] f32 (one contiguous block)
    xy_d = nc.dram_tensor("xy", [128, 2 * BL, GP, D], f32,
                          kind="ExternalInput").ap()
    id_d = nc.dram_tensor("ident", [128, 128], f32, kind="ExternalInput").ap()
    # out: [128, BL*T1 errs (b-major, t-minor)] + [128, 3*BL cost partials]
    OUTC = BL * (T1 + 3)
    out_d = nc.dram_tensor("out_all", [128, OUTC], f32,
                           kind="ExternalOutput").ap()

    SCH_SCALE = float((2.0 / EPS) * 128.0 * LOG2E)
    SCH_BIAS = float(127 * 128)

    eng_of = None  # set inside

    with tile.TileContext(nc) as tc, \
            tc.tile_pool(name="pers", bufs=1) as _pers, \
            tc.tile_pool(name="smtmp", bufs=4) as tmp_pool, \
            tc.tile_pool(name="mm_ps", bufs=3, space="PSUM") as mm_pool, \
            tc.tile_pool(name="q_ps", bufs=1, space="PSUM") as q_pool, \
            tc.tile_pool(name="vec_ps", bufs=1, space="PSUM") as vec_pool, \
            tc.tile_pool(name="sm", bufs=24) as sm, \
            tc.tile_pool(name="smb", bufs=3) as smb:

        def T(shape, dtype, name):
            return _pers.tile(shape, dtype, tag=name, name=name)

        def eng(c):
            return {"A": nc.scalar, "V": nc.vector, "P": nc.gpsimd}[c]

        xin = T([128, 2 * BL, GP, D], f32, name="xin")
        nc.sync.dma_start(xin[:, 0], xy_d[:, 0])
        ident = T([128, 128], f32, name="ident_sb")
        nc.sync.dma_start(ident[:], id_d[:])
        for s in range(1, 2 * BL):
            nc.sync.dma_start(xin[:, s], xy_d[:, s])
        ident16 = T([128, 128], bf16, name="ident16")
        nc.gpsimd.tensor_copy(ident16[:], ident[:])
        c1e6 = T([128, 1], f32, name="c1e6")
        nc.vector.memset(c1e6[:], 1e-6)
        ones16 = T([128, 1], bf16, name="ones16")
        nc.vector.memset(ones16[:], 1.0)
        clogmu = T([128, 1], f32, name="clogmu")
        nc.vector.memset(clogmu[:], float(LOG_MU))


        out_sb = T([128, OUTC], f32, name="out_sb")
        for _b in range(BL):
            nc.vector.memset(
                out_sb[:, _b * (T1 + 3) + T1 + 2:_b * (T1 + 3) + T1 + 3], 0.0)

        Gt = [T([128, GP, N], bf16, name=f"G_{b}") for b in range(BL)]
        GTt = [T([128, GP, N], bf16, name=f"GT_{b}") for b in range(BL)]
        xsa = [T([128, GP, 128], bf16, name=f"xsa_{b}") for b in range(BL)]
        ysa = [T([128, GP, 128], bf16, name=f"ysa_{b}") for b in range(BL)]
        x2 = [T([128, GP], f32, name=f"x2_{b}") for b in range(BL)]
        y2 = [T([128, GP], f32, name=f"y2_{b}") for b in range(BL)]
        xsaT = [T([128, GP, 128], bf16, name=f"xsaT_{b}") for b in range(BL)]
        s0t = [T([128, GP], f32, name=f"s0_{b}") for b in range(BL)]
        ysaT = [T([128, GP, 128], bf16, name=f"ysaT_{b}") for b in range(BL)]

        def ps_slot():
            return mm_pool.tile([128, N], f32, tag="mmps", name="ps")

        # ---------------- softmax + augmented row blocks -------------------
        # chain: exp(A) -> rowsum(V) -> recip(V) -> norm(V or P) -> PE
        # transposes -> augT copy (A).  x2 comes off the ex branch:
        # sq(P) -> sqred(V); x2 = sqred * recip^2.
        def softmax_block(src_idx, aug_out, sq_out, augT_out, is_y, norm_eng,
                          via_dma):
            ex = exs[src_idx]
            ssum = tmp_pool.tile([128, GP], f32, tag="ssum", name="ssum")
            rec = tmp_pool.tile([128, GP], f32, tag="rec", name="rec")
            nc.vector.reduce_sum(ssum[:], ex[:], axis=X)
            nc.vector.reciprocal(rec[:], ssum[:])
            eng(norm_eng).tensor_mul(aug_out[:, :, 0:D], ex[:],
                                     rec[:].broadcast_to([128, GP, D]))
            sq = tmp_pool.tile([128, GP, D], f32, tag="sq", name="sq")
            nc.gpsimd.tensor_mul(sq[:], ex[:], ex[:])
            sqe = tmp_pool.tile([128, GP], f32, tag="sqe", name="sqe")
            nc.vector.reduce_sum(sqe[:], sq[:], axis=X)
            r2 = tmp_pool.tile([128, GP], f32, tag="r2", name="r2")
            nc.vector.tensor_mul(r2[:], rec[:], rec[:])
            nc.vector.tensor_mul(sq_out[:], sqe[:], r2[:])
            nc.gpsimd.memset(aug_out[:, :, 64:128], 0.0)
            neg_col = 65 if is_y else 64
            one_col = 64 if is_y else 65
            nc.vector.tensor_scalar_mul(aug_out[:, :, neg_col], sq_out[:],
                                        -0.5)
            nc.vector.memset(aug_out[:, :, one_col], 1.0)
            if src_idx % BL == 0:
                slot = ps_slot()
                tp = slot[:, 0:512].bitcast(bf16)
                for g in range(GP):
                    nc.tensor.transpose(tp[:, g * 128:(g + 1) * 128],
                                        aug_out[:, g, 0:128],
                                        ident16[:, :])
                tview = tp[:].rearrange("p (g c) -> p g c", g=GP)
                if is_y:
                    nc.vector.tensor_copy(augT_out[:], tview)
                else:
                    nc.scalar.copy(augT_out[:], tview)
            else:
                deferred_augT.append((augT_out, aug_out))


        deferred_augT = []
        # emit the softmax exps for all tensors first (ACT queue front)
        exs = {}
        for s in range(2 * BL):
            ex = tmp_pool.tile([128, GP, D], f32, tag="ex", name=f"ex{s}")
            nc.scalar.activation(ex[:], xin[:, s], AF.Exp)
            exs[s] = ex
        for b in range(BL):
            ne = "V" if b == 0 else "P"
            softmax_block(b, xsa[b], x2[b], xsaT[b], False, ne, True)
            softmax_block(BL + b, ysa[b], y2[b], ysaT[b], True, ne, True)

        def ln_of(s_col, name):
            l = sm.tile([128, GP], f32, tag="l", name=f"{name}_l")
            nc.scalar.activation(l[:], s_col[:], AF.Ln, bias=c1e6[:])
            return l

        late_err = []

        def err_out(l, col, name, defer=False):
            if defer:
                late_err.append((l, col, name))
                return
            dl = sm.tile([128, GP], f32, tag="dl", name=f"{name}_dl")
            nc.vector.tensor_scalar_add(dl[:], l[:], -LOG_MU)
            nc.vector.reduce_sum(out_sb[:, col:col + 1], dl[:], axis=X,
                                 apply_absolute_value=True)

        def flush_late_err():
            for l, col, name in late_err:
                dl = sm.tile([128, GP], f32, tag="dl", name=f"{name}_dl")
                nc.vector.tensor_scalar_add(dl[:], l[:], -LOG_MU)
                nc.vector.reduce_sum(out_sb[:, col:col + 1], dl[:], axis=X,
                                     apply_absolute_value=True)
            late_err.clear()

        def recip16(s_col, name, prev=None, f32_out=True):
            # bf16 (and optionally f32) of prev * MU/(s+1e-6)
            sp = sm.tile([128, GP], f32, tag="sp", name=f"{name}_sp")
            nc.vector.tensor_scalar(sp[:], s_col[:], 1e-6, float(1.0 / MU),
                                    op0=ALU.add, op1=ALU.mult)
            h = sm.tile([128, GP], bf16, tag="hbf16", name=f"{name}_h")
            if prev is None and not f32_out:
                with nc.allow_low_precision("state only feeds matvecs/cost"):
                    nc.vector.reciprocal(h[:], sp[:])
                return h, None
            rc = sm.tile([128, GP], f32, tag="rc", name=f"{name}_rc")
            nc.vector.reciprocal(rc[:], sp[:])
            if prev is None:
                nc.vector.tensor_copy(h[:], rc[:])
                return h, rc
            nc.vector.tensor_mul(h[:], prev[:], rc[:])
            return h, None

        def matvec(dst_ps, state16, kmat):
            for ic in range(GP):
                for jc in range(GP):
                    nc.tensor.matmul(
                        dst_ps[:, ic:ic + 1],
                        lhsT=kmat[:, jc, ic * 128:(ic + 1) * 128],
                        rhs=state16[:, jc:jc + 1],
                        start=(jc == 0), stop=(jc == GP - 1))

        # ---------------- build G + exp + GT + pipelined iterations --------
        # Per-batch iteration stages; stage k of batch b is emitted at a
        # chosen (b', ic) position of the build loop so its ACT/DVE ops land
        # early in those engines' queues (b0's chain hides under b1's exps).
        st = {b: {} for b in range(BL)}

        def S_f1(b):
            l0 = ln_of(s0t[b], f"l0_{b}")
            err_out(l0, b * (T1 + 3) + 0, f"e0_{b}", defer=(b == BL - 1))
            st[b]["f1"], st[b]["f1f"] = recip16(s0t[b], f"f1_{b}")
        def S_sig0(b):
            matvec(mvp[b][:, 0:GP], st[b]["f1"], Gt[b])
        def S_g1(b):
            st[b]["g1"], _ = recip16(mvp[b][:, 0:GP], f"g1_{b}",
                                      f32_out=False)
        def S_s1mv(b):
            matvec(mvp[b][:, GP:2 * GP], st[b]["g1"], GTt[b])
        def S_f2(b):
            s1 = sm.tile([128, GP], f32, tag="s1", name=f"s1_{b}")
            nc.vector.tensor_mul(s1[:], st[b]["f1f"][:], mvp[b][:, GP:2 * GP])
            l1 = ln_of(s1, f"l1_{b}")
            err_out(l1, b * (T1 + 3) + 1, f"e1_{b}", defer=(b == BL - 1))
            st[b]["f2"], _ = recip16(s1, f"f2_{b}", st[b]["f1f"])
        def S_xaug(b):
            f2 = st[b]["f2"]
            xaug = smb.tile([128, GP, D], bf16, tag="xaug", name=f"xaug_{b}")
            nc.vector.tensor_mul(xaug[:], xsa[b][:, :, 0:D],
                                 f2[:].broadcast_to([128, GP, D]))
            fx = sm.tile([128, GP], bf16, tag="fx2", name=f"fx2_{b}")
            nc.vector.tensor_mul(fx[:], x2[b][:], f2[:])
            st[b]["xaug"], st[b]["fx2"] = xaug, fx
        def S_sig1mv(b):
            matvec(mvp[b][:, 2 * GP:3 * GP], st[b]["f2"], Gt[b])
            matvec(mvp[b][:, 3 * GP:4 * GP], st[b]["fx2"], Gt[b])
        def S_qmv(b):
            qq = q_pool.tile([128, GP, D], f32, tag="qq", name=f"qq_{b}")
            for jc in range(GP):
                for gi in range(GP):
                    nc.tensor.matmul(
                        qq[:, jc, :],
                        lhsT=Gt[b][:, gi, jc * 128:(jc + 1) * 128],
                        rhs=st[b]["xaug"][:, gi, :],
                        start=(gi == 0), stop=(gi == GP - 1))
            st[b]["qq"] = qq
        def S_gf(b):
            sig1 = mvp[b][:, 2 * GP:3 * GP]
            if T1 >= 2:
                sg = sm.tile([128, GP], f32, tag="sg", name=f"sg_{b}")
                nc.vector.tensor_mul(sg[:], st[b]["g1"][:], sig1)
                gf, _ = recip16(sg, f"gf_{b}", st[b]["g1"])
            else:
                gf, _ = recip16(sig1, f"gf_{b}")
            st[b]["gf"] = gf
        def S_cost(b):
            sig1 = mvp[b][:, 2 * GP:3 * GP]
            gf = st[b]["gf"]
            yg = smb.tile([128, GP, D], bf16, tag="yg", name=f"yg_{b}")
            nc.vector.tensor_mul(yg[:], ysa[b][:, :, 0:D],
                                 gf[:].broadcast_to([128, GP, D]))
            rjv = smb.tile([128, GP, D], f32, tag="rjv", name=f"rjv_{b}")
            c0 = b * (T1 + 3) + T1
            nc.vector.tensor_mul(rjv[:], st[b]["qq"][:], yg[:])
            nc.vector.reduce_sum(out_sb[:, c0 + 1:c0 + 2],
                                 rjv[:].rearrange("p g d -> p (g d)"),
                                 axis=X)
            w = sm.tile([128, GP], f32, tag="w", name=f"w_{b}")
            nc.vector.tensor_mul(w[:], y2[b][:], sig1)
            nc.vector.tensor_add(w[:], w[:], mvp[b][:, 3 * GP:4 * GP])
            dcost = sm.tile([128, GP], f32, tag="dcost", name=f"dcost_{b}")
            nc.vector.tensor_mul(dcost[:], gf[:], w[:])
            nc.vector.reduce_sum(out_sb[:, c0:c0 + 1], dcost[:], axis=X)
            if b == 0:
                nc.sync.dma_start(out_d[:, 0:T1 + 3], out_sb[:, 0:T1 + 3])

        if T1 >= 2:
            STAGES = [S_f1, S_sig0, S_g1, S_s1mv, S_f2, S_xaug, S_sig1mv,
                      S_qmv, S_gf, S_cost]
        else:
            STAGES = [S_f1, S_xaug, S_sig1mv, S_qmv, S_gf, S_cost]

        # schedule: batch b's stage k emitted at build position (b+1, POS[k])
        # (stage 0 fires at (b, GP-1)); batches beyond the last run post-loop.
        POS = [0, 0, 1, 2, 3, 4, 5, 6, 7, 7] if T1 >= 2 else [0, 2, 3, 4, 5, 6]
        sched = {}
        for b in range(BL):
            for k, fn in enumerate(STAGES):
                if k == 0:
                    key = (b, GP - 1)
                elif b + 1 < BL:
                    key = (b + 1, POS[k])
                else:
                    key = None  # post-loop
                sched.setdefault(key, []).append((fn, b))

        mvp_all = vec_pool.tile([128, 2 * 5 * GP], f32, tag="mvp", name="mvp")
        mvp = {b: mvp_all[:, b * 5 * GP:(b + 1) * 5 * GP] for b in range(BL)}

        for b in range(BL):
            for ic in range(GP):
                ps = ps_slot()
                for h in range(2):
                    nc.tensor.matmul(
                        ps[:, h * 512:(h + 1) * 512],
                        lhsT=xsaT[b][0:66, ic, :],
                        rhs=ysaT[b][0:66].rearrange("p g c -> p (g c)")[:, h * 512:(h + 1) * 512],
                        start=True, stop=True)
                dst = Gt[b][:, ic, :]
                e = EXP_ASSIGN[b * GP + ic]
                if e == "A":
                    nc.scalar.activation(dst, ps[:], AF.Exp,
                                         scale=float(2.0 / EPS),
                                         accum_out=s0t[b][:, ic:ic + 1])
                else:
                    dst_i16 = dst.bitcast(i16)
                    eng(e).tensor_scalar(dst_i16, ps[:], SCH_SCALE,
                                         SCH_BIAS, op0=ALU.mult,
                                         op1=ALU.add)
                    nc.vector.reduce_sum(s0t[b][:, ic:ic + 1], dst, axis=X)
                if b == 0 and ic == 4:
                    for augT_out, aug_out in deferred_augT:
                        nc.sync.dma_start_transpose(
                            augT_out[:],
                            aug_out[:].rearrange("p g c -> p (g c)"))
                    deferred_augT = []
                gdst = GTt[b][:, :, ic * 128:(ic + 1) * 128]
                how = GT_ASSIGN[b * GP + ic]
                if how == "D":
                    nc.sync.dma_start_transpose(gdst, dst)
                else:
                    slot = ps_slot()
                    tpg = slot[:, 0:512].bitcast(bf16)
                    for jc in range(GP):
                        nc.tensor.transpose(
                            tpg[:, jc * 128:(jc + 1) * 128],
                            dst[:, jc * 128:(jc + 1) * 128], ident16[:, :])
                    cc = GT_COPY[(b * GP + ic) % len(GT_COPY)]
                    tview = tpg[:].rearrange("p (g c) -> p g c", g=GP)
                    if cc == "A":
                        nc.scalar.copy(gdst, tview)
                    else:
                        eng(cc).tensor_copy(gdst, tview)
                for fn, bb in sched.get((b, ic), []):
                    fn(bb)
        for fn, bb in sched.get(None, []):
            fn(bb)

        cB = (BL - 1) * (T1 + 3)
        nc.sync.dma_start(out_d[:, cB + T1:], out_sb[:, cB + T1:])
        flush_late_err()
        nc.sync.dma_start(out_d[:, cB:cB + T1], out_sb[:, cB:cB + T1])

    return nc


def _make_runner(nc):
    """Build a cached jitted SPMD callable (one trace+compile per process)."""
    import jax
    import jax.numpy as jnp  # noqa: F401
    from jax.experimental.shard_map import shard_map
    from jax.sharding import Mesh, PartitionSpec

    from concourse import bass2jax, mybir

    bass2jax.install_neuronx_cc_hook()
    assert nc.dbg_addr is None

    partition_name = (nc.partition_id_tensor.name
                      if nc.partition_id_tensor else None)
    in_names, out_names, out_avals, zero_outs = [], [], [], []
    for alloc in nc.m.functions[0].allocations:
        if not isinstance(alloc, mybir.MemoryLocationSet):
            continue
        name = alloc.memorylocations[0].name
        if alloc.kind == "ExternalInput":
            if name != partition_name:
                in_names.append(name)
        elif alloc.kind == "ExternalOutput":
            shape = tuple(alloc.tensor_shape)
            dtype = mybir.dt.np(alloc.dtype)
            out_names.append(name)
            out_avals.append(jax.core.ShapedArray(shape, dtype))
            zero_outs.append(np.zeros(shape, dtype))
    n_params = len(in_names)
    n_outs = len(out_avals)
    all_in_names = in_names + out_names
    if partition_name is not None:
        all_in_names = all_in_names + [partition_name]

    def _body(*args):
        operands = list(args)
        if partition_name is not None:
            operands.append(bass2jax.partition_id_tensor())
        outs = bass2jax._bass_exec_p.bind(
            *operands,
            out_avals=tuple(out_avals),
            in_names=tuple(all_in_names),
            out_names=tuple(out_names),
            lowering_input_output_aliases=(),
            sim_require_finite=True,
            sim_require_nnan=True,
            nc=nc,
        )
        return tuple(outs)

    devices = jax.devices()[:NCORES]
    mesh = Mesh(np.asarray(devices), ("core",))
    in_specs = (PartitionSpec("core"),) * (n_params + n_outs)
    out_specs = (PartitionSpec("core"),) * n_outs
    donate = tuple(range(n_params, n_params + n_outs))
    sharded = jax.jit(
        shard_map(_body, mesh=mesh, in_specs=in_specs, out_specs=out_specs,
                  check_rep=False),
        donate_argnums=donate, keep_unused=True)

    def run(in_maps):
        concat_in = [
            np.concatenate([np.asarray(m[nm]) for m in in_maps], axis=0)
            for nm in in_names
        ]
        concat_zeros = [
            np.zeros((NCORES * z.shape[0], *z.shape[1:]), z.dtype)
            for z in zero_outs
        ]
        out_arrs = sharded(*concat_in, *concat_zeros)
        return [
            {nm: np.asarray(out_arrs[i]).reshape(NCORES, *out_avals[i].shape)[c]
             for i, nm in enumerate(out_names)}
            for c in range(NCORES)
        ]

    return run


def _get_cached(T1: int):
    if T1 not in _CACHE:
        nc = _build_program(T1)
        _CACHE[T1] = (nc, _make_runner(nc))
    return _CACHE[T1]


def _make_in_maps(x: np.ndarray, y: np.ndarray):
    ident = np.eye(128, dtype=np.float32)
    # pack [s, n, d] -> [p, s, g, d] with n = g*128 + p, contiguous per core
    xs = x.reshape(NCORES, BL, GP, 128, D)
    ys = y.reshape(NCORES, BL, GP, 128, D)
    return [{"xy": np.ascontiguousarray(
                 np.concatenate([xs[c], ys[c]], axis=0)   # [2BL, g, p, d]
                 .transpose(2, 0, 1, 3)),                 # [p, 2BL, g, d]
             "ident": ident} for c in range(NCORES)]


def _run_T(T1: int, in_maps):
    _, run = _get_cached(T1)
    results = run(in_maps)
    errs = np.zeros(T1, dtype=np.float64)
    cost_sum = 0.0
    for c in range(NCORES):
        oa = results[c]["out_all"].astype(np.float64)
        for b in range(BL):
            for t in range(T1):
                errs[t] += EPS * oa[:, b * T1 + t].sum()
            c0 = b * (T1 + 3) + T1
            cost_sum += (oa[:, c0].sum()
                         - 2.0 * (oa[:, c0 + 1].sum() + oa[:, c0 + 2].sum()))
    errs /= B
    cost = cost_sum / B
    return errs, cost


def _fallback_reference(x, y):
    """Exact reference semantics, jax op-by-op (slow; only for inputs whose
    Sinkhorn loop doesn't stop after exactly 1-2 iterations)."""
    import jax
    import jax.numpy as jnp

    xs = jax.nn.softmax(jnp.asarray(x), axis=-1)
    ys = jax.nn.softmax(jnp.asarray(y), axis=-1)
    x2 = (xs * xs).sum(-1)
    y2 = (ys * ys).sum(-1)
    xy = jnp.einsum("bid,bjd->bij", xs, ys)
    C = x2[..., :, None] + y2[..., None, :] - 2.0 * xy
    n = xs.shape[-2]
    log_mu = jnp.log(1.0 / n + 1e-8)
    u = jnp.zeros((xs.shape[0], n), dtype=C.dtype)
    v = jnp.zeros_like(u)
    it = 0
    err = np.inf
    while it < MAX_ITER and err >= THRESH:
        u1 = u
        M = (-C + u[..., :, None] + v[..., None, :]) / EPS
        u = EPS * (log_mu - jnp.log(jnp.exp(M).sum(-1) + 1e-6)) + u
        M = (-C + u[..., :, None] + v[..., None, :]) / EPS
        v = EPS * (log_mu - jnp.log(jnp.exp(M).sum(-2) + 1e-6)) + v
        err = float(jnp.abs(u - u1).sum(-1).mean())
        it += 1
    M = (-C + u[..., :, None] + v[..., None, :]) / EPS
    pi = jnp.exp(M)
    cost = (pi * C).sum((-2, -1))
    return np.float32(np.asarray(cost.mean()))


def kernel(x: np.ndarray, y: np.ndarray) -> np.ndarray:
    x = np.asarray(x, dtype=np.float32)
    y = np.asarray(y, dtype=np.float32)
    assert x.shape == (B, N, D) and y.shape == (B, N, D)
    in_maps = _make_in_maps(x, y)

    errs, cost = _run_T(2, in_maps)
    if errs[0] >= THRESH and errs[1] < THRESH:
        return np.float32(cost)
    if errs[0] < THRESH:
        _, cost1 = _run_T(1, in_maps)
        return np.float32(cost1)
    return _fallback_reference(x, y)


if __name__ == "__main__":
    from concourse.timeline_sim import TimelineSim
    nc = _build_program(2)
    tl = TimelineSim(nc)
    print(f"TimelineSim: {tl.simulate():.0f} ns")



# revision 44
# speedup vs baseline: 1.0065x; 1.0065x over previous
"""Sinkhorn distance (entropic OT) on 8 Trainium2 NeuronCores — v2.

Data-parallel over batch: B=16 -> 2 batches/core.  Per batch:

  G = exp(-C/eps) built directly on PE via an augmented 66-row contraction
  (softmaxed x,y transposed + the -x2/2 / -y2/2 folding rows), so the
  multiplicative Sinkhorn scalings start at f0 = g0 = 1 and the first
  row-sum s0 = G@1 falls out of the exp pass (ACT accumulator / an extra
  row-sum for Schraudolph chunks).

  exp is split across engines: ACT computes exact exp chunks (with free
  accum_out row sums); DVE/Pool chunks use a Schraudolph exp (one
  tensor_scalar f32->int16 pass bitcast to bf16; max entry err ~4%, final
  cost err ~5e-4, validated offline).

  GT (needed only for the s1 = G@g1 matvec) is produced by DMA xbar
  transposes (SP engine HWDGE; free of compute engines) and/or PE
  transposes (split tunable).

  All Sinkhorn state lives in column layout [128, 8].  Matvecs are
  tall-skinny accumulating matmuls (dst [128, 1]; the big matrix is the
  stationary operand), so each full matvec costs ~64 tiny instructions
  instead of a 3.4us wide stream.  The final cost contracts through one
  [128 j, 66] Q stream that also yields sigma1 (merging the last
  v-update), then small fused multiply-reduces.

Sync legality: this walrus build accepts only 1 semaphore wait per
instruction.  Instead of manual observe/claim gymnastics, a generic
legalizer wraps TileContext._commit_and_lower and splits any multi-wait
instruction into single-wait EventSemaphore instructions on the same
engine queue ahead of it.
"""

import sys

sys.path.insert(0, "/opt/trn_rl_repo")

import numpy as np

EPS = 0.1
THRESH = 0.1
MAX_ITER = 100
B, N, D = 16, 1024, 64
NCORES = 8
BL = B // NCORES
GP = 8
MU = float(np.float32(1.0 / N + 1e-8))
LOG_MU = float(np.log(np.float32(1.0 / N + 1e-8), dtype=np.float32))
LOG2E = 1.4426950408889634

_CACHE: dict = {}
_PATCHED = [False]

# ---------------- engine assignment knobs (tuned against TimelineSim) ----
# exp engine per (b, ic) chunk index 0..15: 'A' = ACT exact, 'V' = DVE
# Schraudolph, 'P' = Pool Schraudolph.
EXP_ASSIGN = "AAVAAVAA" + "AAVAAVAA"
# s0 row-sum engine for non-ACT chunks: 'V' or 'P'
S0_ASSIGN = {"V": "P", "P": "V"}
# GT stripe (b, g) 0..15: 'D' = DMA transpose, 'E' = PE transpose
GT_ASSIGN = "DDDDDDDD" + "DDDDDDDD"
# PE-built GT stripes: PSUM->SBUF copy engine cycle
GT_COPY = "AV"
# chain-stage emission positions within the next batch's build loop
POS10 = [0, 0, 1, 2, 3, 4, 5, 6, 7, 7]
# prioritize batch-0 tensors in the input DMA / softmax-exp emission order
REORDER = True
# defer the last batch's err outputs to the kernel tail (True) or emit
# inline right after f1/f2 (False)
DEFER_ERR = True
# issue some input DMAs on the ACT HWDGE queue to parallelize the load
INPUT_SPLIT = False
# s0 row-sum engine for Schraudolph chunks: 'V' = DVE reduce, 'A' = ACT
# Copy+accum per (b, ic) chunk; only consulted for non-'A' EXP chunks
S0_ENG = "VVVVVVVV" + "VVVVVVVV"


def _patch_tile():
    """1) split multi-wait instructions into single-wait EventSemaphores;
    2) split the kernel-tail drain the same way (as the v1 kernel did)."""
    if _PATCHED[0]:
        return
    import concourse.tile as tile
    from concourse import mybir
    from concourse.vector_clock import ScopedClock

    _orig_cal = tile.TileContext._commit_and_lower

    def _cal(self, inst, original_block, old_bb_map, bb_to_exit_bb):
        si = inst.sync_info
        if (si is not None and si.on_wait and len(si.on_wait) > 1
                and inst.engine != mybir.EngineType.Unassigned):
            waits = list(si.on_wait)
            eng = self.nc.engines[inst.engine]
            by_name = {s.name: s for s in self.sems.allocated().values()}
            kept = []
            for w in waits[:-1]:
                sem = by_name.get(w.ant_name)
                if sem is None:
                    kept.append(w)
                    continue
                eng.wait_ge(sem, w.wait_value)
            si.on_wait = kept + [waits[-1]]
        return _orig_cal(self, inst, original_block, old_bb_map, bb_to_exit_bb)

    tile.TileContext._commit_and_lower = _cal

    def _drain_and_barrier(self, tick_clock, wait_clock):
        nc = self.nc
        drain_inst = nc.sync.drain()
        wait_clock.add_sem_waits(
            drain_inst.ins, ScopedClock({None: tick_clock.global_clock}))
        waits = list(drain_inst.ins.sync_info.on_wait or [])
        if len(waits) > 1:
            drain_inst.ins.sync_info.on_wait = waits[:1]
            by_name = {s.name: s for s in self.sems.allocated().values()}
            for w in waits[1:]:
                d2 = nc.sync.drain()
                d2._wait_ge(by_name[w.ant_name], w.wait_value)
        nc.all_engine_barrier()
        assert self.sems is not None
        popped = nc._tile_sem_poison_stack.pop()
        assert popped is self._sem_poison
        nc.clear_and_free_semaphores(list(self.sems.allocated().values()))
        nc.all_engine_barrier()

    tile.TileContext._drain_and_barrier = _drain_and_barrier
    _PATCHED[0] = True


HEAT_N = 0


def _build_program(T1: int):
    import concourse.bass as bass
    import concourse.tile as tile
    from concourse import mybir

    _patch_tile()

    f32 = mybir.dt.float32
    bf16 = mybir.dt.bfloat16
    i16 = mybir.dt.int16
    AF = mybir.ActivationFunctionType
    X = mybir.AxisListType.X
    ALU = mybir.AluOpType

    nc = bass.Bass("TRN2", target_bir_lowering=False, debug=False,
                   num_devices=NCORES, num_swdge_queues=1)

    # inputs: host-prepacked [128, 2BL, GP, D] f32 (one contiguous block)
    xy_d = nc.dram_tensor("xy", [128, 2 * BL, GP, D], f32,
                          kind="ExternalInput").ap()
    id_d = nc.dram_tensor("ident", [128, 128], f32, kind="ExternalInput").ap()
    # out: [128, BL*T1 errs (b-major, t-minor)] + [128, 3*BL cost partials]
    OUTC = BL * (T1 + 3)
    out_d = nc.dram_tensor("out_all", [128, OUTC], f32,
                           kind="ExternalOutput").ap()

    SCH_SCALE = float((2.0 / EPS) * 128.0 * LOG2E)
    SCH_BIAS = float(127 * 128)

    eng_of = None  # set inside

    with tile.TileContext(nc) as tc, \
            tc.tile_pool(name="pers", bufs=1) as _pers, \
            tc.tile_pool(name="smtmp", bufs=4) as tmp_pool, \
            tc.tile_pool(name="mm_ps", bufs=3, space="PSUM") as mm_pool, \
            tc.tile_pool(name="q_ps", bufs=1, space="PSUM") as q_pool, \
            tc.tile_pool(name="vec_ps", bufs=1, space="PSUM") as vec_pool, \
            tc.tile_pool(name="sm", bufs=24) as sm, \
            tc.tile_pool(name="smb", bufs=3) as smb:

        def T(shape, dtype, name):
            return _pers.tile(shape, dtype, tag=name, name=name)

        def eng(c):
            return {"A": nc.scalar, "V": nc.vector, "P": nc.gpsimd}[c]

        xin = T([128, 2 * BL, GP, D], f32, name="xin")
        if REORDER:
            dma_order = [0, BL] + [s for s in range(2 * BL)
                                   if s not in (0, BL)]
        else:
            dma_order = list(range(2 * BL))
        def in_eng(i):
            return nc.scalar if (INPUT_SPLIT and i % 2 == 1) else nc.sync
        in_eng(0).dma_start(xin[:, dma_order[0]], xy_d[:, dma_order[0]])
        ident = T([128, 128], f32, name="ident_sb")
        nc.sync.dma_start(ident[:], id_d[:])
        for i, s in enumerate(dma_order[1:]):
            in_eng(i + 1).dma_start(xin[:, s], xy_d[:, s])
        ident16 = T([128, 128], bf16, name="ident16")
        nc.gpsimd.tensor_copy(ident16[:], ident[:])
        c1e6 = T([128, 1], f32, name="c1e6")
        nc.vector.memset(c1e6[:], 1e-6)
        ones16 = T([128, 1], bf16, name="ones16")
        nc.vector.memset(ones16[:], 1.0)
        clogmu = T([128, 1], f32, name="clogmu")
        nc.vector.memset(clogmu[:], float(LOG_MU))


        out_sb = T([128, OUTC], f32, name="out_sb")
        for _b in range(BL):
            nc.vector.memset(
                out_sb[:, _b * (T1 + 3) + T1 + 2:_b * (T1 + 3) + T1 + 3], 0.0)

        Gt = [T([128, GP, N], bf16, name=f"G_{b}") for b in range(BL)]
        GTt = [T([128, GP, N], bf16, name=f"GT_{b}") for b in range(BL)]
        xsa = [T([128, GP, 128], bf16, name=f"xsa_{b}") for b in range(BL)]
        ysa = [T([128, GP, 128], bf16, name=f"ysa_{b}") for b in range(BL)]
        x2 = [T([128, GP], f32, name=f"x2_{b}") for b in range(BL)]
        y2 = [T([128, GP], f32, name=f"y2_{b}") for b in range(BL)]
        xsaT = [T([128, GP, 128], bf16, name=f"xsaT_{b}") for b in range(BL)]
        s0t = [T([128, GP], f32, name=f"s0_{b}") for b in range(BL)]
        ysaT = [T([128, GP, 128], bf16, name=f"ysaT_{b}") for b in range(BL)]

        def ps_slot():
            return mm_pool.tile([128, N], f32, tag="mmps", name="ps")

        # ---------------- softmax + augmented row blocks -------------------
        # chain: exp(A) -> rowsum(V) -> recip(V) -> norm(V or P) -> PE
        # transposes -> augT copy (A).  x2 comes off the ex branch:
        # sq(P) -> sqred(V); x2 = sqred * recip^2.
        def softmax_block(src_idx, aug_out, sq_out, augT_out, is_y, norm_eng,
                          via_dma):
            ex = exs[src_idx]
            ssum = tmp_pool.tile([128, GP], f32, tag="ssum", name="ssum")
            rec = tmp_pool.tile([128, GP], f32, tag="rec", name="rec")
            nc.vector.reduce_sum(ssum[:], ex[:], axis=X)
            nc.vector.reciprocal(rec[:], ssum[:])
            eng(norm_eng).tensor_mul(aug_out[:, :, 0:D], ex[:],
                                     rec[:].broadcast_to([128, GP, D]))
            sq = tmp_pool.tile([128, GP, D], f32, tag="sq", name="sq")
            nc.gpsimd.tensor_mul(sq[:], ex[:], ex[:])
            sqe = tmp_pool.tile([128, GP], f32, tag="sqe", name="sqe")
            nc.vector.reduce_sum(sqe[:], sq[:], axis=X)
            r2 = tmp_pool.tile([128, GP], f32, tag="r2", name="r2")
            nc.vector.tensor_mul(r2[:], rec[:], rec[:])
            nc.vector.tensor_mul(sq_out[:], sqe[:], r2[:])
            nc.gpsimd.memset(aug_out[:, :, 64:128], 0.0)
            neg_col = 65 if is_y else 64
            one_col = 64 if is_y else 65
            nc.vector.tensor_scalar_mul(aug_out[:, :, neg_col], sq_out[:],
                                        -0.5)
            nc.vector.memset(aug_out[:, :, one_col], 1.0)
            if src_idx % BL == 0:
                slot = ps_slot()
                tp = slot[:, 0:512].bitcast(bf16)
                for g in range(GP):
                    nc.tensor.transpose(tp[:, g * 128:(g + 1) * 128],
                                        aug_out[:, g, 0:128],
                                        ident16[:, :])
                tview = tp[:].rearrange("p (g c) -> p g c", g=GP)
                if is_y:
                    nc.vector.tensor_copy(augT_out[:], tview)
                else:
                    nc.scalar.copy(augT_out[:], tview)
            else:
                deferred_augT.append((augT_out, aug_out))


        deferred_augT = []
        # emit the softmax exps for all tensors first (ACT queue front)
        exs = {}
        for s in dma_order:
            ex = tmp_pool.tile([128, GP, D], f32, tag="ex", name=f"ex{s}")
            nc.scalar.activation(ex[:], xin[:, s], AF.Exp)
            exs[s] = ex
        for b in range(BL):
            ne = "V" if b == 0 else "P"
            softmax_block(b, xsa[b], x2[b], xsaT[b], False, ne, True)
            softmax_block(BL + b, ysa[b], y2[b], ysaT[b], True, ne, True)

        def ln_of(s_col, name):
            l = sm.tile([128, GP], f32, tag="l", name=f"{name}_l")
            nc.scalar.activation(l[:], s_col[:], AF.Ln, bias=c1e6[:])
            return l

        late_err = []

        def err_out(l, col, name, defer=False):
            if defer and DEFER_ERR:
                late_err.append((l, col, name))
                return
            dl = sm.tile([128, GP], f32, tag="dl", name=f"{name}_dl")
            nc.vector.tensor_scalar_add(dl[:], l[:], -LOG_MU)
            nc.vector.reduce_sum(out_sb[:, col:col + 1], dl[:], axis=X,
                                 apply_absolute_value=True)

        def flush_late_err():
            for l, col, name in late_err:
                dl = sm.tile([128, GP], f32, tag="dl", name=f"{name}_dl")
                nc.vector.tensor_scalar_add(dl[:], l[:], -LOG_MU)
                nc.vector.reduce_sum(out_sb[:, col:col + 1], dl[:], axis=X,
                                     apply_absolute_value=True)
            late_err.clear()

        def recip16(s_col, name, prev=None, f32_out=True):
            # bf16 (and optionally f32) of prev * MU/(s+1e-6)
            sp = sm.tile([128, GP], f32, tag="sp", name=f"{name}_sp")
            nc.vector.tensor_scalar(sp[:], s_col[:], 1e-6, float(1.0 / MU),
                                    op0=ALU.add, op1=ALU.mult)
            h = sm.tile([128, GP], bf16, tag="hbf16", name=f"{name}_h")
            if prev is None and not f32_out:
                with nc.allow_low_precision("state only feeds matvecs/cost"):
                    nc.vector.reciprocal(h[:], sp[:])
                return h, None
            rc = sm.tile([128, GP], f32, tag="rc", name=f"{name}_rc")
            nc.vector.reciprocal(rc[:], sp[:])
            if prev is None:
                nc.vector.tensor_copy(h[:], rc[:])
                return h, rc
            nc.vector.tensor_mul(h[:], prev[:], rc[:])
            return h, None

        def matvec(dst_ps, state16, kmat):
            for ic in range(GP):
                for jc in range(GP):
                    nc.tensor.matmul(
                        dst_ps[:, ic:ic + 1],
                        lhsT=kmat[:, jc, ic * 128:(ic + 1) * 128],
                        rhs=state16[:, jc:jc + 1],
                        start=(jc == 0), stop=(jc == GP - 1))

        # ---------------- build G + exp + GT + pipelined iterations --------
        # Per-batch iteration stages; stage k of batch b is emitted at a
        # chosen (b', ic) position of the build loop so its ACT/DVE ops land
        # early in those engines' queues (b0's chain hides under b1's exps).
        st = {b: {} for b in range(BL)}

        def S_f1(b):
            l0 = ln_of(s0t[b], f"l0_{b}")
            err_out(l0, b * (T1 + 3) + 0, f"e0_{b}", defer=(b == BL - 1))
            st[b]["f1"], st[b]["f1f"] = recip16(s0t[b], f"f1_{b}")
            if T1 < 2:
                st[b]["f2"] = st[b]["f1"]
        def S_sig0(b):
            matvec(mvp[b][:, 0:GP], st[b]["f1"], Gt[b])
        def S_g1(b):
            st[b]["g1"], _ = recip16(mvp[b][:, 0:GP], f"g1_{b}",
                                      f32_out=False)
        def S_s1mv(b):
            matvec(mvp[b][:, GP:2 * GP], st[b]["g1"], GTt[b])
        def S_f2(b):
            s1 = sm.tile([128, GP], f32, tag="s1", name=f"s1_{b}")
            nc.vector.tensor_mul(s1[:], st[b]["f1f"][:], mvp[b][:, GP:2 * GP])
            l1 = ln_of(s1, f"l1_{b}")
            err_out(l1, b * (T1 + 3) + 1, f"e1_{b}", defer=(b == BL - 1))
            st[b]["f2"], _ = recip16(s1, f"f2_{b}", st[b]["f1f"])
        def S_xaug(b):
            f2 = st[b]["f2"]
            xaug = smb.tile([128, GP, D], bf16, tag="xaug", name=f"xaug_{b}")
            nc.vector.tensor_mul(xaug[:], xsa[b][:, :, 0:D],
                                 f2[:].broadcast_to([128, GP, D]))
            # 2-col state [f2*x2 | f2]: one 2-col matvec then yields both
            # qx2 = G^T(f2*x2) and (softmax rows sum to 1) sig1 = G^T f2
            xx2 = sm.tile([128, GP, 2], bf16, tag="fx2", name=f"fx2_{b}")
            nc.vector.tensor_mul(xx2[:, :, 0], x2[b][:], f2[:])
            nc.vector.tensor_copy(xx2[:, :, 1], f2[:])
            st[b]["xaug"], st[b]["xx2"] = xaug, xx2
        def S_sig1mv(b):
            # qx2/sig1 interleaved into mvp cols 2GP+[0,1]+2*ic
            for ic in range(GP):
                for jc in range(GP):
                    nc.tensor.matmul(
                        mvp[b][:, 2 * GP + 2 * ic:2 * GP + 2 * ic + 2],
                        lhsT=Gt[b][:, jc, ic * 128:(ic + 1) * 128],
                        rhs=st[b]["xx2"][:, jc, :],
                        start=(jc == 0), stop=(jc == GP - 1))
        def S_qmv(b):
            qq = q_pool.tile([128, GP, D], f32, tag="qq", name=f"qq_{b}")
            for jc in range(GP):
                for gi in range(GP):
                    nc.tensor.matmul(
                        qq[:, jc, :],
                        lhsT=Gt[b][:, gi, jc * 128:(jc + 1) * 128],
                        rhs=st[b]["xaug"][:, gi, :],
                        start=(gi == 0), stop=(gi == GP - 1))
            st[b]["qq"] = qq
        def qs_view(b):
            return mvp[b][:, 2 * GP:4 * GP].rearrange(
                "p (g two) -> p g two", two=2)
        def S_gf(b):
            sig1 = qs_view(b)[:, :, 1]
            if T1 >= 2:
                sg = sm.tile([128, GP], f32, tag="sg", name=f"sg_{b}")
                nc.vector.tensor_mul(sg[:], st[b]["g1"][:], sig1)
                gf, _ = recip16(sg, f"gf_{b}", st[b]["g1"])
            else:
                gf, _ = recip16(sig1, f"gf_{b}")
            st[b]["gf"] = gf
        def S_cost(b):
            qs = qs_view(b)
            gf = st[b]["gf"]
            yg = smb.tile([128, GP, D], bf16, tag="yg", name=f"yg_{b}")
            nc.vector.tensor_mul(yg[:], ysa[b][:, :, 0:D],
                                 gf[:].broadcast_to([128, GP, D]))
            rjv = smb.tile([128, GP, D], f32, tag="rjv", name=f"rjv_{b}")
            c0 = b * (T1 + 3) + T1
            nc.vector.tensor_mul(rjv[:], st[b]["qq"][:], yg[:])
            nc.vector.reduce_sum(out_sb[:, c0 + 1:c0 + 2],
                                 rjv[:].rearrange("p g d -> p (g d)"),
                                 axis=X)
            w = sm.tile([128, GP], f32, tag="w", name=f"w_{b}")
            nc.vector.tensor_mul(w[:], y2[b][:], qs[:, :, 1])
            nc.vector.tensor_add(w[:], w[:], qs[:, :, 0])
            dcost = sm.tile([128, GP], f32, tag="dcost", name=f"dcost_{b}")
            nc.vector.tensor_mul(dcost[:], gf[:], w[:])
            nc.vector.reduce_sum(out_sb[:, c0:c0 + 1], dcost[:], axis=X)
            if b == 0:
                nc.sync.dma_start(out_d[:, 0:T1 + 3], out_sb[:, 0:T1 + 3])

        if T1 >= 2:
            STAGES = [S_f1, S_sig0, S_g1, S_s1mv, S_f2, S_xaug, S_sig1mv,
                      S_qmv, S_gf, S_cost]
        else:
            STAGES = [S_f1, S_xaug, S_sig1mv, S_qmv, S_gf, S_cost]

        # schedule: batch b's stage k emitted at build position (b+1, POS[k])
        # (stage 0 fires at (b, GP-1)); batches beyond the last run post-loop.
        POS = POS10 if T1 >= 2 else [0, 2, 3, 4, 5, 6]
        sched = {}
        for b in range(BL):
            for k, fn in enumerate(STAGES):
                if k == 0:
                    key = (b, GP - 1)
                elif b + 1 < BL:
                    key = (b + 1, POS[k])
                else:
                    key = None  # post-loop
                sched.setdefault(key, []).append((fn, b))

        mvp_all = vec_pool.tile([128, 2 * 5 * GP], f32, tag="mvp", name="mvp")
        mvp = {b: mvp_all[:, b * 5 * GP:(b + 1) * 5 * GP] for b in range(BL)}

        for b in range(BL):
            for ic in range(GP):
                ps = ps_slot()
                for h in range(2):
                    nc.tensor.matmul(
                        ps[:, h * 512:(h + 1) * 512],
                        lhsT=xsaT[b][0:66, ic, :],
                        rhs=ysaT[b][0:66].rearrange("p g c -> p (g c)")[:, h * 512:(h + 1) * 512],
                        start=True, stop=True)
                dst = Gt[b][:, ic, :]
                e = EXP_ASSIGN[b * GP + ic]
                if e == "A":
                    nc.scalar.activation(dst, ps[:], AF.Exp,
                                         scale=float(2.0 / EPS),
                                         accum_out=s0t[b][:, ic:ic + 1])
                else:
                    dst_i16 = dst.bitcast(i16)
                    eng(e).tensor_scalar(dst_i16, ps[:], SCH_SCALE,
                                         SCH_BIAS, op0=ALU.mult,
                                         op1=ALU.add)
                    if S0_ENG[b * GP + ic] == "A":
                        scr = tmp_pool.tile([128, N], bf16, tag="s0scr",
                                            name="s0scr")
                        nc.scalar.activation(scr[:], dst, AF.Copy,
                                             accum_out=s0t[b][:, ic:ic + 1])
                    else:
                        nc.vector.reduce_sum(s0t[b][:, ic:ic + 1], dst,
                                             axis=X)
                if b == 0 and ic == 4:
                    for augT_out, aug_out in deferred_augT:
                        nc.sync.dma_start_transpose(
                            augT_out[:],
                            aug_out[:].rearrange("p g c -> p (g c)"))
                    deferred_augT = []
                gdst = GTt[b][:, :, ic * 128:(ic + 1) * 128]
                how = GT_ASSIGN[b * GP + ic]
                if how == "D":
                    nc.sync.dma_start_transpose(gdst, dst)
                else:
                    slot = ps_slot()
                    tpg = slot[:, 0:512].bitcast(bf16)
                    for jc in range(GP):
                        nc.tensor.transpose(
                            tpg[:, jc * 128:(jc + 1) * 128],
                            dst[:, jc * 128:(jc + 1) * 128], ident16[:, :])
                    cc = GT_COPY[(b * GP + ic) % len(GT_COPY)]
                    tview = tpg[:].rearrange("p (g c) -> p g c", g=GP)
                    if cc == "A":
                        nc.scalar.copy(gdst, tview)
                    else:
                        eng(cc).tensor_copy(gdst, tview)
                for fn, bb in sched.get((b, ic), []):
                    fn(bb)
        for fn, bb in sched.get(None, []):
            fn(bb)

        cB = (BL - 1) * (T1 + 3)
        nc.sync.dma_start(out_d[:, cB + T1:], out_sb[:, cB + T1:])
        flush_late_err()
        nc.sync.dma_start(out_d[:, cB:cB + T1], out_sb[:, cB:cB + T1])

    return nc


def _make_runner(nc):
    """Build a cached jitted SPMD callable (one trace+compile per process)."""
    import jax
    import jax.numpy as jnp  # noqa: F401
    from jax.experimental.shard_map import shard_map
    from jax.sharding import Mesh, PartitionSpec

    from concourse import bass2jax, mybir

    bass2jax.install_neuronx_cc_hook()
    assert nc.dbg_addr is None

    partition_name = (nc.partition_id_tensor.name
                      if nc.partition_id_tensor else None)
    in_names, out_names, out_avals, zero_outs = [], [], [], []
    for alloc in nc.m.functions[0].allocations:
        if not isinstance(alloc, mybir.MemoryLocationSet):
            continue
        name = alloc.memorylocations[0].name
        if alloc.kind == "ExternalInput":
            if name != partition_name:
                in_names.append(name)
        elif alloc.kind == "ExternalOutput":
            shape = tuple(alloc.tensor_shape)
            dtype = mybir.dt.np(alloc.dtype)
            out_names.append(name)
            out_avals.append(jax.core.ShapedArray(shape, dtype))
            zero_outs.append(np.zeros(shape, dtype))
    n_params = len(in_names)
    n_outs = len(out_avals)
    all_in_names = in_names + out_names
    if partition_name is not None:
        all_in_names = all_in_names + [partition_name]

    def _body(*args):
        operands = list(args)
        if partition_name is not None:
            operands.append(bass2jax.partition_id_tensor())
        outs = bass2jax._bass_exec_p.bind(
            *operands,
            out_avals=tuple(out_avals),
            in_names=tuple(all_in_names),
            out_names=tuple(out_names),
            lowering_input_output_aliases=(),
            sim_require_finite=True,
            sim_require_nnan=True,
            nc=nc,
        )
        return tuple(outs)

    devices = jax.devices()[:NCORES]
    mesh = Mesh(np.asarray(devices), ("core",))
    in_specs = (PartitionSpec("core"),) * (n_params + n_outs)
    out_specs = (PartitionSpec("core"),) * n_outs
    donate = tuple(range(n_params, n_params + n_outs))
    sharded = jax.jit(
        shard_map(_body, mesh=mesh, in_specs=in_specs, out_specs=out_specs,
                  check_rep=False),
        donate_argnums=donate, keep_unused=True)

    def run(in_maps):
        concat_in = [
            np.concatenate([np.asarray(m[nm]) for m in in_maps], axis=0)
            for nm in in_names
        ]
        concat_zeros = [
            np.zeros((NCORES * z.shape[0], *z.shape[1:]), z.dtype)
            for z in zero_outs
        ]
        out_arrs = sharded(*concat_in, *concat_zeros)
        return [
            {nm: np.asarray(out_arrs[i]).reshape(NCORES, *out_avals[i].shape)[c]
             for i, nm in enumerate(out_names)}
            for c in range(NCORES)
        ]

    return run


def _get_cached(T1: int):
    if T1 not in _CACHE:
        nc = _build_program(T1)
        _CACHE[T1] = (nc, _make_runner(nc))
    return _CACHE[T1]


def _make_in_maps(x: np.ndarray, y: np.ndarray):
    ident = np.eye(128, dtype=np.float32)
    # pack [s, n, d] -> [p, s, g, d] with n = g*128 + p, contiguous per core
    xs = x.reshape(NCORES, BL, GP, 128, D)
    ys = y.reshape(NCORES, BL, GP, 128, D)
    return [{"xy": np.ascontiguousarray(
                 np.concatenate([xs[c], ys[c]], axis=0)   # [2BL, g, p, d]
                 .transpose(2, 0, 1, 3)),                 # [p, 2BL, g, d]
             "ident": ident} for c in range(NCORES)]


def _run_T(T1: int, in_maps):
    _, run = _get_cached(T1)
    results = run(in_maps)
    errs = np.zeros(T1, dtype=np.float64)
    cost_sum = 0.0
    for c in range(NCORES):
        oa = results[c]["out_all"].astype(np.float64)
        for b in range(BL):
            for t in range(T1):
                errs[t] += EPS * oa[:, b * T1 + t].sum()
            c0 = b * (T1 + 3) + T1
            cost_sum += (oa[:, c0].sum()
                         - 2.0 * (oa[:, c0 + 1].sum() + oa[:, c0 + 2].sum()))
    errs /= B
    cost = cost_sum / B
    return errs, cost


def _fallback_reference(x, y):
    """Exact reference semantics, jax op-by-op (slow; only for inputs whose
    Sinkhorn loop doesn't stop after exactly 1-2 iterations)."""
    import jax
    import jax.numpy as jnp

    xs = jax.nn.softmax(jnp.asarray(x), axis=-1)
    ys = jax.nn.softmax(jnp.asarray(y), axis=-1)
    x2 = (xs * xs).sum(-1)
    y2 = (ys * ys).sum(-1)
    xy = jnp.einsum("bid,bjd->bij", xs, ys)
    C = x2[..., :, None] + y2[..., None, :] - 2.0 * xy
    n = xs.shape[-2]
    log_mu = jnp.log(1.0 / n + 1e-8)
    u = jnp.zeros((xs.shape[0], n), dtype=C.dtype)
    v = jnp.zeros_like(u)
    it = 0
    err = np.inf
    while it < MAX_ITER and err >= THRESH:
        u1 = u
        M = (-C + u[..., :, None] + v[..., None, :]) / EPS
        u = EPS * (log_mu - jnp.log(jnp.exp(M).sum(-1) + 1e-6)) + u
        M = (-C + u[..., :, None] + v[..., None, :]) / EPS
        v = EPS * (log_mu - jnp.log(jnp.exp(M).sum(-2) + 1e-6)) + v
        err = float(jnp.abs(u - u1).sum(-1).mean())
        it += 1
    M = (-C + u[..., :, None] + v[..., None, :]) / EPS
    pi = jnp.exp(M)
    cost = (pi * C).sum((-2, -1))
    return np.float32(np.asarray(cost.mean()))


def kernel(x: np.ndarray, y: np.ndarray) -> np.ndarray:
    x = np.asarray(x, dtype=np.float32)
    y = np.asarray(y, dtype=np.float32)
    assert x.shape == (B, N, D) and y.shape == (B, N, D)
    in_maps = _make_in_maps(x, y)

    errs, cost = _run_T(2, in_maps)
    if errs[0] >= THRESH and errs[1] < THRESH:
        return np.float32(cost)
    if errs[0] < THRESH:
        _, cost1 = _run_T(1, in_maps)
        return np.float32(cost1)
    return _fallback_reference(x, y)


if __name__ == "__main__":
    from concourse.timeline_sim import TimelineSim
    nc = _build_program(2)
    tl = TimelineSim(nc)
    print(f"TimelineSim: {tl.simulate():.0f} ns")

